# revision 48
# baseline (speedup 1.0000x reference)
"""Single-head causal attention on 8 Trainium2 NeuronCores.

Problem: x[B=8, T=2048, E=1024] fp32, Wq/Wk/Wv [E, H=64] fp32.
    q = x @ Wq; k = x @ Wk; v = x @ Wv
    out = softmax(causal(q @ k^T / sqrt(H))) @ v          -> [8, 2048, 64]

Sharding: pure data parallel, one batch element per core; weights replicated.

Per-core kernel design (transposed-scores formulation):
  - xT[e, t] built from x via PE transpose-mode (fp32, exact) - every
    projection contracts over e, so e must be on the partition axis.
  - One M=128 matmul computes qT and kT together ([Wq|Wk] packed); kT is
    shifted from psum partitions 64:128 down to base-0 partitions with an
    SBUF->SBUF DMA (compute engines cannot cross partitions).
  - vT [64, 2048], then small PE transposes -> vaug[:, j, :] = [v | 1]
    tiles [128, 65]; the ones column makes the softmax denominator fall
    out of the AV matmul for free.
  - scoresT[s, t] = kT_j.T @ qT into PSUM; diagonal blocks get an additive
    -1e30 triangular mask (DVE) before exp(scale*x) on the ACT engine.
    exp without max-subtraction is safe: |scores| <~ 6.  Below-diagonal
    blocks are skipped by narrowing the AV matmul column range instead of
    masking.
  - outT[65, 512] accumulates vaug_j.T @ expT_j over j; row 64 = softmax
    denominator.  Small PE transpose back to [t, 65], multiply rows by the
    reciprocal denominator, DMA out.
  - Heavy matmuls run as float32r (full-rate fp32 streaming, ~tf32-grade:
    measured rel err 2.6e-4 end to end) with an fp32 fallback
    (ATTN_MM_DTYPE=f32, 4 cycles/row).  float32r operands must be produced
    by explicitly-rounding instructions (BIR verifier rule), hence the
    mm_dt-typed tiles; transposes stay fp32 (exact data movement).
  - Groups of 512 t-columns are software-pipelined: attention(g) emission
    interleaves with loads/transposes/projections(g+1), with projection
    matmuls kept in dense runs (transpose-mode does not count as PE-busy
    for the HAM clock gate, so dense matmul runs keep the PE at 2.4 GHz).
"""

import os

import numpy as np

import concourse.bacc as bacc
import concourse.bass as bass
import concourse.tile as tile
from concourse import mybir
from concourse.masks import make_identity

B, T, E, H = 8, 2048, 1024, 64
P = 128                      # SBUF partitions
NE = E // P                  # 8 e-chunks
NT = T // P                  # 16 t-chunks (also s-chunks)
GW = 512                     # t-group width (matmul moving-operand max, fp32)
NG = T // GW                 # 4 t-groups
CPG = GW // P                # 4 chunks per group
F32 = mybir.dt.float32

# Matmul dtype for the heavy matmuls: "f32r" (fast) or "f32" (exact).
MM_DTYPE = os.environ.get("ATTN_MM_DTYPE", "f32r")

_NC_CACHE: dict = {}




def build_attention_nc(mm_dtype: str = "f32r", repeat: int = 1) -> bass.Bass:
    """Build the single-core Bass program (SPMD across cores via in_maps)."""
    mm_dt = {"f32": F32, "f32r": mybir.dt.float32r, "bf16": F32}[mm_dtype]

    nc = bacc.Bacc("TRN2", target_bir_lowering=False, debug=False)
    x_d = nc.dram_tensor("x", [T, E], F32, kind="ExternalInput").ap()
    wq_d = nc.dram_tensor("Wq", [E, H], F32, kind="ExternalInput").ap()
    wk_d = nc.dram_tensor("Wk", [E, H], F32, kind="ExternalInput").ap()
    wv_d = nc.dram_tensor("Wv", [E, H], F32, kind="ExternalInput").ap()
    out_d = nc.dram_tensor("out", [T, H], F32, kind="ExternalOutput").ap()

    with tile.TileContext(nc) as tc:
        with (
            tc.tile_pool(name="const", bufs=1) as const,
            tc.tile_pool(name="xin", bufs=6) as xin,
            tc.tile_pool(name="xt", bufs=NE) as xtp,
            tc.tile_pool(name="proj", bufs=1) as projp,
            tc.tile_pool(name="vaug", bufs=1) as vaugp,
            tc.tile_pool(name="expt", bufs=10) as exptp,
            tc.tile_pool(name="outs", bufs=4) as outsp,
            tc.tile_pool(name="ps_sc", bufs=2, space="PSUM") as ps_sc_p,
            tc.tile_pool(name="ps_tr", bufs=2, space="PSUM") as ps_tr_p,
            tc.tile_pool(name="ps_pm", bufs=2, space="PSUM") as ps_pm_p,
        ):
            # --- constants ---------------------------------------------------
            ident = const.tile([P, P], F32)
            make_identity(nc, ident)
            # Additive causal mask, applied to score PSUM before exp.
            # bigmask[s, u] = -1e30 where u < 384 + s else 0.  For a diagonal
            # j-block at position rel (j = 4g+rel), the slice
            # bigmask[:, 384-rel*128 : 384-rel*128+(rel+1)*128] masks the
            # below-diagonal t-chunks AND the in-block upper triangle.
            bigmask = const.tile([P, GW], F32)
            nc.gpsimd.memset(bigmask, 0.0)
            nc.gpsimd.affine_select(
                out=bigmask, in_=bigmask,
                compare_op=mybir.AluOpType.is_ge,
                fill=-1e30, base=-384,
                pattern=[[1, GW]], channel_multiplier=-1,
            )
            # weights, e-major: [p, c, h] with e = c*128 + p  (SWDGE queue so
            # the x-tile loads own the HWDGE queue from t=0).  Wq and Wk are
            # packed side by side so one M=128 matmul computes both heads'
            # projections: psum rows 0:64 = qT, rows 64:128 = kT.
            wqk_f = const.tile([P, NE, 2 * H], F32, tag="wqkf")
            nc.gpsimd.dma_start(
                out=wqk_f[:, :, :H], in_=wq_d.rearrange("(c p) h -> p c h", p=P))
            nc.gpsimd.dma_start(
                out=wqk_f[:, :, H:], in_=wk_d.rearrange("(c p) h -> p c h", p=P))
            wv_f = const.tile([P, NE, H], F32, tag="wvf")
            nc.gpsimd.dma_start(
                out=wv_f, in_=wv_d.rearrange("(c p) h -> p c h", p=P))
            wqk = const.tile([P, NE, 2 * H], mm_dt, tag="wqk")
            nc.vector.tensor_copy(wqk, wqk_f)
            wv = const.tile([P, NE, H], mm_dt, tag="wv")
            nc.vector.tensor_copy(wv, wv_f)
            ones = const.tile([P, NT, 1], F32, tag="ones")
            nc.gpsimd.memset(ones, 1.0)

            # persistent per-iteration state (allocated fresh each repeat)
            def body(_iv=None):
                xT = [xtp.tile([P, T], mm_dt, tag="xt", name=f"xT{c}") for c in range(NE)]
                qT = projp.tile([H, T], mm_dt, tag="qt")
                kT = projp.tile([H, T], mm_dt, tag="kt")
                # replicas on partitions 64:128 so two K=64 score matmuls can
                # run concurrently in different PE row-groups (tile_position)
                qT2 = projp.tile([P, T], mm_dt, tag="qt2")
                kT2 = projp.tile([P, T], mm_dt, tag="kt2")
                vT = projp.tile([H, T], F32, tag="vt")
                # vaug[s, j, :] = [v | 1] per s-chunk j; ones column via DVE
                # copy (f32r memset fails the walrus ISA check)
                vaug = vaugp.tile([P, NT, H + 1], mm_dt, tag="vaug")
                nc.vector.tensor_copy(vaug[:, :, H:H + 1], ones)

                # one-time absorber: PE picks up the Pool-engine sem for the
                # identity/mask constants ahead of the first transposes
                dmy = ps_tr_p.tile([1, P], F32, tag="tr", name="dmy0")
                nc.tensor.transpose(dmy, ident[:, :1], ident)

                def loads(g):
                    xts = [xin.tile([P, E], F32, tag="xin", name=f"xin{i}")
                           for i in range(CPG)]
                    nq = 4 if g == 0 else 2
                    for q in range(nq):
                        lo, hi = q * E // nq, (q + 1) * E // nq
                        for i in range(CPG):
                            r0 = (g * CPG + i) * P
                            eng = nc.sync if i % 2 == 0 else nc.scalar
                            eng.dma_start(
                                out=xts[i][:, lo:hi], in_=x_d[r0:r0 + P, lo:hi])
                    return xts

                def tp_qk_units(g, xts):
                    """x-transpose + q/k projection for group g (pipeline
                    filler units)."""
                    g0 = g * GW

                    def emit_trb(c):
                        ps = ps_tr_p.tile([P, GW], F32, tag="tr", name=f"trb{c}")
                        for ii in range(CPG):
                            nc.tensor.transpose(
                                ps[:, ii * P:(ii + 1) * P],
                                xts[ii][:, c * P:(c + 1) * P], ident)
                        nc.vector.tensor_copy(xT[c][:, g0:g0 + GW], ps)

                    for c in range(NE):
                        emit_trb(c)
                        yield
                    # dense matmul run (keeps the PE activity monitor warm -
                    # transpose-mode doesn't count as PE-busy for HAM)
                    psqk = ps_pm_p.tile([P, GW], F32, tag="pm", name="psqk")
                    for c in range(NE):
                        nc.tensor.matmul(
                            psqk, wqk[:, c, :], xT[c][:, g0:g0 + GW],
                            start=(c == 0), stop=(c == NE - 1))
                        if c % 2:
                            yield
                    nc.vector.tensor_copy(qT[:, g0:g0 + GW], psqk[:H, :])
                    # kT lands on psum partitions 64:128: keep that replica in
                    # kT2 and DMA-shift it down to base-0 partitions for kT;
                    # qT gets the opposite treatment (DMA-shift up to qT2)
                    nc.vector.tensor_copy(kT2[H:, g0:g0 + GW], psqk[H:, :])
                    nc.sync.dma_start(out=kT[:, g0:g0 + GW], in_=kT2[H:, g0:g0 + GW])
                    nc.sync.dma_start(out=qT2[H:, g0:g0 + GW], in_=qT[:, g0:g0 + GW])
                    yield

                def tp_v_units(g):
                    """v projection + vaug build for group g; independent of
                    the qk phase so it can fill a different window."""
                    g0 = g * GW
                    psp = ps_pm_p.tile([H, GW], F32, tag="pm", name="psp")
                    for c in range(NE):
                        nc.tensor.matmul(
                            psp, wv[:, c, :], xT[c][:, g0:g0 + GW],
                            start=(c == 0), stop=(c == NE - 1))
                        if c % 2:
                            yield
                    nc.vector.tensor_copy(vT[:, g0:g0 + GW], psp)
                    yield
                    # vaug[:, j, :64] = v rows for this group's s-chunks
                    psv = ps_tr_p.tile([P, CPG, H], F32, tag="tr", name="psv")
                    for ii in range(CPG):
                        nc.tensor.transpose(
                            psv[:, ii, :],
                            vT[:, (g * CPG + ii) * P:(g * CPG + ii + 1) * P],
                            ident[:H, :H])
                    nc.vector.tensor_copy(
                        vaug[:, g * CPG:(g + 1) * CPG, :H], psv)
                    # absorber: surface the vaug-copy DVE dep on PE before the
                    # AV matmuls
                    dmyg = ps_tr_p.tile([1, P], F32, tag="tr", name=f"dmy{g}")
                    nc.tensor.transpose(
                        dmyg, vaug[:, g * CPG, :1].bitcast(F32), ident)
                    yield

                def attn_units(g):
                    """scores -> exp -> AV -> normalize for group g."""
                    g0 = g * GW
                    ps_av = ps_pm_p.tile([H + 1, GW], F32, tag="pm", name="ps_av")
                    njb = CPG * (g + 1)          # j-blocks 0 .. 4g+3
                    ets = []

                    def emit_av(m):
                        et_m = ets[m]
                        for hf in range(2):
                            j = 2 * m + hf
                            rel = max(j - CPG * g, 0)
                            nc.tensor.matmul(
                                ps_av[:, rel * P:],
                                vaug[:, j, :],
                                et_m[:, hf * GW + rel * P:(hf + 1) * GW],
                                start=(j == 0), stop=(j == njb - 1))

                    for m in range(njb // 2):
                        ps_s = ps_sc_p.tile([P, 2 * GW], F32, tag="sc")
                        for hf in range(2):
                            j = 2 * m + hf
                            if hf == 0:
                                lhsT, rhs = kT[:, j * P:(j + 1) * P], qT[:, g0:g0 + GW]
                            else:
                                lhsT = kT2[H:, j * P:(j + 1) * P]
                                rhs = qT2[H:, g0:g0 + GW]
                            nc.tensor.matmul(
                                ps_s[:, hf * GW:(hf + 1) * GW],
                                lhsT, rhs,
                                start=True, stop=True)
                            rel = j - CPG * g
                            if rel >= 0:
                                # diagonal block: mask only the triangle; the
                                # below-diagonal columns are never read (the
                                # AV matmul is narrowed past them)
                                reg = ps_s[:, hf * GW + rel * P:hf * GW + (rel + 1) * P]
                                nc.vector.tensor_add(
                                    reg, reg, bigmask[:, 384:384 + P])
                        et = exptp.tile([P, 2 * GW], mm_dt, tag="expt")
                        if m == njb // 2 - 1 and njb >= 4:
                            # last pair: j = 4g+2, 4g+3 -> AV only reads cols
                            # >= 256 of each half; skip exp on the dead half
                            nc.scalar.activation(
                                et.rearrange("p (h w) -> p h w", h=2)[:, :, GW // 2:],
                                ps_s.rearrange("p (h w) -> p h w", h=2)[:, :, GW // 2:],
                                mybir.ActivationFunctionType.Exp,
                                scale=float(H) ** -0.5)
                        else:
                            nc.scalar.activation(
                                et, ps_s, mybir.ActivationFunctionType.Exp,
                                scale=float(H) ** -0.5)
                        ets.append(et)
                        yield
                        if m >= 1:
                            emit_av(m - 1)
                            yield
                    emit_av(njb // 2 - 1)
                    yield

                    # normalize + write out
                    avT = outsp.tile([H + 1, GW], F32, tag="avt")
                    if g == NG - 1:
                        for ii in range(CPG):
                            nc.vector.tensor_copy(
                                avT[:, ii * P:(ii + 1) * P],
                                ps_av[:, ii * P:(ii + 1) * P])
                    else:
                        nc.vector.tensor_copy(avT, ps_av)
                    for ii in range(CPG):
                        i = g * CPG + ii
                        ps_o = ps_tr_p.tile([P, H + 1], F32, tag="tr", name="ps_o")
                        nc.tensor.transpose(
                            ps_o, avT[:, ii * P:(ii + 1) * P],
                            ident[:H + 1, :H + 1])
                        rcp = outsp.tile([P, 1], F32, tag="rcp")
                        nc.vector.reciprocal(rcp, ps_o[:, H:H + 1])
                        ot = outsp.tile([P, H], F32, tag="ot")
                        nc.vector.tensor_scalar_mul(ot, ps_o[:, :H], rcp)
                        nc.gpsimd.dma_start(
                            out=out_d[i * P:(i + 1) * P, :], in_=ot)
                        yield

                # software pipeline: attention(g) interleaves with
                # loads/transposes/projections of group g+1.  The final
                # group's v-phase is deferred into attention(NG-1)'s window,
                # which otherwise has no filler work.
                import itertools as _it
                done = object()
                for _ in _it.chain(tp_qk_units(0, loads(0)), tp_v_units(0)):
                    pass
                for g in range(NG):
                    gen_att = attn_units(g)
                    if g + 1 < NG:
                        chain = [tp_qk_units(g + 1, loads(g + 1))]
                        if g + 1 < NG - 1:
                            chain.append(tp_v_units(g + 1))
                        gen_tp = _it.chain(*chain)
                    elif g == NG - 1:
                        gen_tp = tp_v_units(NG - 1)
                    while True:
                        a = next(gen_att, done)
                        t = next(gen_tp, done) if gen_tp is not None else done
                        if a is done and t is done:
                            break

            if repeat == 1:
                body()
            else:
                tc.For_i_unrolled_general(
                    0, repeat, 1,
                    lambda iv0, unroll: body(iv0), 1,
                    hint_engines=(
                        mybir.EngineType.PE, mybir.EngineType.DVE,
                        mybir.EngineType.Activation, mybir.EngineType.SP,
                        mybir.EngineType.Pool))

    nc.compile()
    return nc


class _Runner:
    """Cached jitted SPMD executor for one built nc.

    run_bass_kernel_spmd rebuilds jax.jit(shard_map(...)) on every call,
    which forces a full XLA retrace + NEFF reload each time.  Building the
    jitted callable once (and keeping inputs device-resident) turns repeat
    calls from ~1.4 s into milliseconds, which the timing harness needs.
    """

    def __init__(self, nc):
        import jax
        from jax.experimental.shard_map import shard_map
        from jax.sharding import Mesh, NamedSharding, PartitionSpec
        from concourse import bass2jax, mybir as mb

        bass2jax.install_neuronx_cc_hook()
        in_names, out_names, out_avals = [], [], []
        for alloc in nc.m.functions[0].allocations:
            if not isinstance(alloc, mb.MemoryLocationSet):
                continue
            name = alloc.memorylocations[0].name
            if alloc.kind == "ExternalInput":
                in_names.append(name)
            elif alloc.kind == "ExternalOutput":
                out_names.append(name)
                out_avals.append(jax.core.ShapedArray(
                    tuple(alloc.tensor_shape), mb.dt.np(alloc.dtype)))
        assert nc.dbg_addr is None
        part_name = nc.partition_id_tensor.name if nc.partition_id_tensor else None
        if part_name is not None:
            in_names = [n for n in in_names if n != part_name]
        self.in_names, self.out_names, self.out_avals = in_names, out_names, out_avals
        n_params = len(in_names)
        all_names = in_names + out_names
        if part_name is not None:
            all_names = all_names + [part_name]

        def _body(*args):
            operands = list(args)
            if part_name is not None:
                operands.append(bass2jax.partition_id_tensor())
            outs = bass2jax._bass_exec_p.bind(
                *operands,
                out_avals=tuple(out_avals),
                in_names=tuple(all_names),
                out_names=tuple(out_names),
                lowering_input_output_aliases=(),
                sim_require_finite=True,
                sim_require_nnan=True,
                nc=nc,
            )
            return tuple(outs)

        devices = jax.devices()[:B]
        self.mesh = Mesh(np.asarray(devices), ("core",))
        self.spec = PartitionSpec("core")
        self.sharding = NamedSharding(self.mesh, self.spec)
        nin = n_params + len(out_names)
        self.fn = jax.jit(
            shard_map(
                _body, mesh=self.mesh,
                in_specs=(self.spec,) * nin,
                out_specs=(self.spec,) * len(out_names),
                check_rep=False,
            ),
            donate_argnums=tuple(range(n_params, nin)),
            keep_unused=True,
        )
        self._dev_inputs = {}

    def prep_inputs(self, in_maps, cache_key=None):
        """Concat per-core inputs to global arrays, optionally device-cached."""
        import jax
        if cache_key is not None and cache_key in self._dev_inputs:
            return self._dev_inputs[cache_key]
        concat = [
            np.concatenate([np.asarray(m[n]) for m in in_maps], axis=0)
            for n in self.in_names
        ]
        arrs = [jax.device_put(a, self.sharding) for a in concat]
        jax.block_until_ready(arrs)
        if cache_key is not None:
            self._dev_inputs[cache_key] = arrs
        return arrs

    def __call__(self, dev_inputs, block=True):
        import jax
        zeros = [
            np.zeros((B * av.shape[0], *av.shape[1:]), av.dtype)
            for av in self.out_avals
        ]
        outs = self.fn(*dev_inputs, *zeros)
        if block:
            jax.block_until_ready(outs)
        return outs

    def gather(self, outs):
        o = np.asarray(outs[0])
        return o.reshape(B, -1, o.shape[-1])


def _get_runner(mm_dtype: str, repeat: int) -> "_Runner":
    key = (mm_dtype, repeat)
    if key not in _NC_CACHE:
        _NC_CACHE[key] = _Runner(build_attention_nc(mm_dtype, repeat))
    return _NC_CACHE[key]


def _make_in_maps(inputs: dict):
    x = np.asarray(inputs["x"], dtype=np.float32)
    wq = np.ascontiguousarray(np.asarray(inputs["Wq"], dtype=np.float32))
    wk = np.ascontiguousarray(np.asarray(inputs["Wk"], dtype=np.float32))
    wv = np.ascontiguousarray(np.asarray(inputs["Wv"], dtype=np.float32))
    return [
        {"x": np.ascontiguousarray(x[i]), "Wq": wq, "Wk": wk, "Wv": wv}
        for i in range(B)
    ]


def run_spmd(inputs: dict, mm_dtype: str = MM_DTYPE, repeat: int = 1,
             cache_key=None):
    r = _get_runner(mm_dtype, repeat)
    dev = r.prep_inputs(_make_in_maps(inputs), cache_key=cache_key)
    return r.gather(r(dev))


def kernel(**inputs) -> np.ndarray:
    return run_spmd(inputs, MM_DTYPE, repeat=1)


# revision 49
# speedup vs baseline: 1.0516x; 1.0516x over previous
"""Single-head causal attention on 8 Trainium2 NeuronCores.

Problem: x[B=8, T=2048, E=1024] fp32, Wq/Wk/Wv [E, H=64] fp32.
    q = x @ Wq; k = x @ Wk; v = x @ Wv
    out = softmax(causal(q @ k^T / sqrt(H))) @ v          -> [8, 2048, 64]

Sharding: pure data parallel, one batch element per core; weights replicated.

Per-core kernel design (transposed-scores formulation):
  - xT[e, t] built from x via PE transpose-mode (fp32, exact) - every
    projection contracts over e, so e must be on the partition axis.
  - One M=128 matmul computes qT and kT together ([Wq|Wk] packed); kT is
    shifted from psum partitions 64:128 down to base-0 partitions with an
    SBUF->SBUF DMA (compute engines cannot cross partitions).
  - vT [64, 2048], then small PE transposes -> vaug[:, j, :] = [v | 1]
    tiles [128, 65]; the ones column makes the softmax denominator fall
    out of the AV matmul for free.
  - scoresT[s, t] = kT_j.T @ qT into PSUM; diagonal blocks get an additive
    -1e30 triangular mask (DVE) before exp(scale*x) on the ACT engine.
    exp without max-subtraction is safe: |scores| <~ 6.  Below-diagonal
    blocks are skipped by narrowing the AV matmul column range instead of
    masking.
  - outT[65, 512] accumulates vaug_j.T @ expT_j over j; row 64 = softmax
    denominator.  Small PE transpose back to [t, 65], multiply rows by the
    reciprocal denominator, DMA out.
  - Heavy matmuls run as float32r (full-rate fp32 streaming, ~tf32-grade:
    measured rel err 2.6e-4 end to end) with an fp32 fallback
    (ATTN_MM_DTYPE=f32, 4 cycles/row).  float32r operands must be produced
    by explicitly-rounding instructions (BIR verifier rule), hence the
    mm_dt-typed tiles; transposes stay fp32 (exact data movement).
  - Groups of 512 t-columns are software-pipelined: attention(g) emission
    interleaves with loads/transposes/projections(g+1), with projection
    matmuls kept in dense runs (transpose-mode does not count as PE-busy
    for the HAM clock gate, so dense matmul runs keep the PE at 2.4 GHz).
"""

import os

import numpy as np

import concourse.bacc as bacc
import concourse.bass as bass
import concourse.tile as tile
from concourse import mybir
from concourse.masks import make_identity

B, T, E, H = 8, 2048, 1024, 64
P = 128                      # SBUF partitions
NE = E // P                  # 8 e-chunks
NT = T // P                  # 16 t-chunks (also s-chunks)
GW = 512                     # t-group width (matmul moving-operand max, fp32)
NG = T // GW                 # 4 t-groups
CPG = GW // P                # 4 chunks per group
F32 = mybir.dt.float32

# Matmul dtype for the heavy matmuls: "f32r" (fast) or "f32" (exact).
MM_DTYPE = os.environ.get("ATTN_MM_DTYPE", "f32r")

_NC_CACHE: dict = {}




def build_attention_nc(mm_dtype: str = "f32r", repeat: int = 1) -> bass.Bass:
    """Build the single-core Bass program (SPMD across cores via in_maps)."""
    mm_dt = {"f32": F32, "f32r": mybir.dt.float32r, "bf16": F32}[mm_dtype]

    nc = bacc.Bacc("TRN2", target_bir_lowering=False, debug=False)
    x_d = nc.dram_tensor("x", [T, E], F32, kind="ExternalInput").ap()
    wq_d = nc.dram_tensor("Wq", [E, H], F32, kind="ExternalInput").ap()
    wk_d = nc.dram_tensor("Wk", [E, H], F32, kind="ExternalInput").ap()
    wv_d = nc.dram_tensor("Wv", [E, H], F32, kind="ExternalInput").ap()
    out_d = nc.dram_tensor("out", [T, H], F32, kind="ExternalOutput").ap()

    with tile.TileContext(nc) as tc:
        with (
            tc.tile_pool(name="const", bufs=1) as const,
            tc.tile_pool(name="xin", bufs=6) as xin,
            tc.tile_pool(name="xt", bufs=NE) as xtp,
            tc.tile_pool(name="proj", bufs=1) as projp,
            tc.tile_pool(name="vaug", bufs=1) as vaugp,
            tc.tile_pool(name="expt", bufs=10) as exptp,
            tc.tile_pool(name="outs", bufs=4) as outsp,
            tc.tile_pool(name="ps_sc", bufs=2, space="PSUM") as ps_sc_p,
            tc.tile_pool(name="ps_tr", bufs=2, space="PSUM") as ps_tr_p,
            tc.tile_pool(name="ps_pm", bufs=2, space="PSUM") as ps_pm_p,
        ):
            # --- constants ---------------------------------------------------
            ident = const.tile([P, P], F32)
            make_identity(nc, ident)
            # Additive causal mask, applied to score PSUM before exp.
            # bigmask[s, u] = -1e30 where u < 384 + s else 0.  For a diagonal
            # j-block at position rel (j = 4g+rel), the slice
            # bigmask[:, 384-rel*128 : 384-rel*128+(rel+1)*128] masks the
            # below-diagonal t-chunks AND the in-block upper triangle.
            bigmask = const.tile([P, GW], F32)
            nc.gpsimd.memset(bigmask, 0.0)
            nc.gpsimd.affine_select(
                out=bigmask, in_=bigmask,
                compare_op=mybir.AluOpType.is_ge,
                fill=-1e30, base=-384,
                pattern=[[1, GW]], channel_multiplier=-1,
            )
            # weights, e-major: [p, c, h] with e = c*128 + p  (SWDGE queue so
            # the x-tile loads own the HWDGE queue from t=0).  Wq and Wk are
            # packed side by side so one M=128 matmul computes both heads'
            # projections: psum rows 0:64 = qT, rows 64:128 = kT.
            wqk_f = const.tile([P, NE, 2 * H], F32, tag="wqkf")
            nc.gpsimd.dma_start(
                out=wqk_f[:, :, :H], in_=wq_d.rearrange("(c p) h -> p c h", p=P))
            nc.gpsimd.dma_start(
                out=wqk_f[:, :, H:], in_=wk_d.rearrange("(c p) h -> p c h", p=P))
            wv_f = const.tile([P, NE, H], F32, tag="wvf")
            nc.gpsimd.dma_start(
                out=wv_f, in_=wv_d.rearrange("(c p) h -> p c h", p=P))
            wqk = const.tile([P, NE, 2 * H], mm_dt, tag="wqk")
            nc.vector.tensor_copy(wqk, wqk_f)
            wv = const.tile([P, NE, H], mm_dt, tag="wv")
            nc.vector.tensor_copy(wv, wv_f)
            ones = const.tile([P, NT, 1], F32, tag="ones")
            nc.gpsimd.memset(ones, 1.0)

            # persistent per-iteration state (allocated fresh each repeat)
            def body(_iv=None, staged=False):
                xT = [xtp.tile([P, T], mm_dt, tag="xt", name=f"xT{c}") for c in range(NE)]
                qT = projp.tile([H, T], mm_dt, tag="qt")
                kT = projp.tile([H, T], mm_dt, tag="kt")
                # replicas on partitions 64:128 so two K=64 score matmuls can
                # run concurrently in different PE row-groups (tile_position)
                qT2 = projp.tile([P, T], mm_dt, tag="qt2")
                kT2 = projp.tile([P, T], mm_dt, tag="kt2")
                vT = projp.tile([H, T], F32, tag="vt")
                # vaug[s, j, :] = [v | 1] per s-chunk j; ones column via DVE
                # copy (f32r memset fails the walrus ISA check)
                vaug = vaugp.tile([P, NT, H + 1], mm_dt, tag="vaug")
                nc.vector.tensor_copy(vaug[:, :, H:H + 1], ones)

                # one-time absorber: PE picks up the Pool-engine sem for the
                # identity/mask constants ahead of the first transposes
                dmy = ps_tr_p.tile([1, P], F32, tag="tr", name="dmy0")
                nc.tensor.transpose(dmy, ident[:, :1], ident)

                def loads(g):
                    xts = [xin.tile([P, E], F32, tag="xin", name=f"xin{i}")
                           for i in range(CPG)]
                    nq = 4 if g == 0 else 2
                    for q in range(nq):
                        lo, hi = q * E // nq, (q + 1) * E // nq
                        for i in range(CPG):
                            r0 = (g * CPG + i) * P
                            eng = nc.sync if i % 2 == 0 else nc.scalar
                            eng.dma_start(
                                out=xts[i][:, lo:hi], in_=x_d[r0:r0 + P, lo:hi])
                    return xts

                def tp_qk_units(g, xts):
                    """x-transpose + q/k projection for group g (pipeline
                    filler units)."""
                    g0 = g * GW

                    def emit_trb(c):
                        ps = ps_tr_p.tile([P, GW], F32, tag="tr", name=f"trb{c}")
                        for ii in range(CPG):
                            nc.tensor.transpose(
                                ps[:, ii * P:(ii + 1) * P],
                                xts[ii][:, c * P:(c + 1) * P], ident)
                        nc.vector.tensor_copy(xT[c][:, g0:g0 + GW], ps)

                    for c in range(NE):
                        emit_trb(c)
                        yield
                    # dense matmul run (keeps the PE activity monitor warm -
                    # transpose-mode doesn't count as PE-busy for HAM)
                    psqk = ps_pm_p.tile([P, GW], F32, tag="pm", name="psqk")
                    for c in range(NE):
                        nc.tensor.matmul(
                            psqk, wqk[:, c, :], xT[c][:, g0:g0 + GW],
                            start=(c == 0), stop=(c == NE - 1))
                        if c % 2:
                            yield
                    nc.vector.tensor_copy(qT[:, g0:g0 + GW], psqk[:H, :])
                    # kT lands on psum partitions 64:128: keep that replica in
                    # kT2 and DMA-shift it down to base-0 partitions for kT;
                    # qT gets the opposite treatment (DMA-shift up to qT2)
                    nc.vector.tensor_copy(kT2[H:, g0:g0 + GW], psqk[H:, :])
                    nc.sync.dma_start(out=kT[:, g0:g0 + GW], in_=kT2[H:, g0:g0 + GW])
                    nc.sync.dma_start(out=qT2[H:, g0:g0 + GW], in_=qT[:, g0:g0 + GW])
                    yield

                def tp_v_units(g):
                    """v projection + vaug build for group g; independent of
                    the qk phase so it can fill a different window."""
                    g0 = g * GW
                    psp = ps_pm_p.tile([H, GW], F32, tag="pm", name="psp")
                    for c in range(NE):
                        nc.tensor.matmul(
                            psp, wv[:, c, :], xT[c][:, g0:g0 + GW],
                            start=(c == 0), stop=(c == NE - 1))
                        if c % 2:
                            yield
                    nc.vector.tensor_copy(vT[:, g0:g0 + GW], psp)
                    yield
                    # vaug[:, j, :64] = v rows for this group's s-chunks
                    psv = ps_tr_p.tile([P, CPG, H], F32, tag="tr", name="psv")
                    for ii in range(CPG):
                        nc.tensor.transpose(
                            psv[:, ii, :],
                            vT[:, (g * CPG + ii) * P:(g * CPG + ii + 1) * P],
                            ident[:H, :H])
                    nc.vector.tensor_copy(
                        vaug[:, g * CPG:(g + 1) * CPG, :H], psv)
                    # absorber: surface the vaug-copy DVE dep on PE before the
                    # AV matmuls
                    dmyg = ps_tr_p.tile([1, P], F32, tag="tr", name=f"dmy{g}")
                    nc.tensor.transpose(
                        dmyg, vaug[:, g * CPG, :1].bitcast(F32), ident)
                    yield

                def attn_units(g):
                    """scores -> exp -> AV -> normalize for group g."""
                    g0 = g * GW
                    ps_av = ps_pm_p.tile([H + 1, GW], F32, tag="pm", name="ps_av")
                    njb = CPG * (g + 1)          # j-blocks 0 .. 4g+3
                    ets = []

                    def emit_av(m):
                        et_m = ets[m]
                        for hf in range(2):
                            j = 2 * m + hf
                            rel = max(j - CPG * g, 0)
                            nc.tensor.matmul(
                                ps_av[:, rel * P:],
                                vaug[:, j, :],
                                et_m[:, hf * GW + rel * P:(hf + 1) * GW],
                                start=(j == 0), stop=(j == njb - 1))

                    for m in range(njb // 2):
                        ps_s = ps_sc_p.tile([P, 2 * GW], F32, tag="sc")
                        for hf in range(2):
                            j = 2 * m + hf
                            if hf == 0:
                                lhsT, rhs = kT[:, j * P:(j + 1) * P], qT[:, g0:g0 + GW]
                            else:
                                lhsT = kT2[H:, j * P:(j + 1) * P]
                                rhs = qT2[H:, g0:g0 + GW]
                            nc.tensor.matmul(
                                ps_s[:, hf * GW:(hf + 1) * GW],
                                lhsT, rhs,
                                start=True, stop=True)
                            rel = j - CPG * g
                            if rel >= 0:
                                # diagonal block: mask only the triangle; the
                                # below-diagonal columns are never read (the
                                # AV matmul is narrowed past them)
                                reg = ps_s[:, hf * GW + rel * P:hf * GW + (rel + 1) * P]
                                nc.vector.tensor_add(
                                    reg, reg, bigmask[:, 384:384 + P])
                        et = exptp.tile([P, 2 * GW], mm_dt, tag="expt")
                        if m == njb // 2 - 1 and njb >= 4:
                            # last pair: j = 4g+2, 4g+3 -> AV only reads cols
                            # >= 256 of each half; skip exp on the dead half
                            nc.scalar.activation(
                                et.rearrange("p (h w) -> p h w", h=2)[:, :, GW // 2:],
                                ps_s.rearrange("p (h w) -> p h w", h=2)[:, :, GW // 2:],
                                mybir.ActivationFunctionType.Exp,
                                scale=float(H) ** -0.5)
                        else:
                            nc.scalar.activation(
                                et, ps_s, mybir.ActivationFunctionType.Exp,
                                scale=float(H) ** -0.5)
                        ets.append(et)
                        yield
                        if m >= 1:
                            emit_av(m - 1)
                            yield
                    emit_av(njb // 2 - 1)
                    yield

                    # normalize + write out
                    avT = outsp.tile([H + 1, GW], F32, tag="avt")
                    if g == NG - 1:
                        for ii in range(CPG):
                            nc.vector.tensor_copy(
                                avT[:, ii * P:(ii + 1) * P],
                                ps_av[:, ii * P:(ii + 1) * P])
                    else:
                        nc.vector.tensor_copy(avT, ps_av)
                    for ii in range(CPG):
                        i = g * CPG + ii
                        ps_o = ps_tr_p.tile([P, H + 1], F32, tag="tr", name="ps_o")
                        nc.tensor.transpose(
                            ps_o, avT[:, ii * P:(ii + 1) * P],
                            ident[:H + 1, :H + 1])
                        rcp = outsp.tile([P, 1], F32, tag="rcp")
                        nc.vector.reciprocal(rcp, ps_o[:, H:H + 1])
                        ot = outsp.tile([P, H], F32, tag="ot")
                        nc.vector.tensor_scalar_mul(ot, ps_o[:, :H], rcp)
                        nc.gpsimd.dma_start(
                            out=out_d[i * P:(i + 1) * P, :], in_=ot)
                        yield

                # software pipeline: attention(g) interleaves with
                # loads/transposes/projections of group g+1.  The final
                # group's v-phase is deferred into attention(NG-1)'s window,
                # which otherwise has no filler work.
                import itertools as _it
                done = object()
                for _ in _it.chain(tp_qk_units(0, loads(0)), tp_v_units(0)):
                    pass
                for g in range(NG):
                    gen_att = attn_units(g)
                    if g + 1 < NG:
                        chain = [tp_qk_units(g + 1, loads(g + 1))]
                        if g + 1 < NG - 1:
                            chain.append(tp_v_units(g + 1))
                        gen_tp = _it.chain(*chain)
                    elif g == NG - 1:
                        gen_tp = tp_v_units(NG - 1)
                    while True:
                        a = next(gen_att, done)
                        t = next(gen_tp, done) if gen_tp is not None else done
                        if a is done and t is done:
                            break

            if repeat == 1:
                body()
            else:
                # staggered_reset overlaps the loop's semaphore resets with
                # compute instead of a full drain + all-engine barrier per
                # back edge; stage boundaries fall on the 4 group windows
                with tc.For_i(
                        0, repeat, 1,
                        staggered_reset=True,
                        hint_engines=(
                            mybir.EngineType.PE, mybir.EngineType.DVE,
                            mybir.EngineType.Activation, mybir.EngineType.SP,
                            mybir.EngineType.Pool)):
                    body(staged=True)

    nc.compile()
    return nc


class _Runner:
    """Cached jitted SPMD executor for one built nc.

    run_bass_kernel_spmd rebuilds jax.jit(shard_map(...)) on every call,
    which forces a full XLA retrace + NEFF reload each time.  Building the
    jitted callable once (and keeping inputs device-resident) turns repeat
    calls from ~1.4 s into milliseconds, which the timing harness needs.
    """

    def __init__(self, nc):
        import jax
        from jax.experimental.shard_map import shard_map
        from jax.sharding import Mesh, NamedSharding, PartitionSpec
        from concourse import bass2jax, mybir as mb

        bass2jax.install_neuronx_cc_hook()
        in_names, out_names, out_avals = [], [], []
        for alloc in nc.m.functions[0].allocations:
            if not isinstance(alloc, mb.MemoryLocationSet):
                continue
            name = alloc.memorylocations[0].name
            if alloc.kind == "ExternalInput":
                in_names.append(name)
            elif alloc.kind == "ExternalOutput":
                out_names.append(name)
                out_avals.append(jax.core.ShapedArray(
                    tuple(alloc.tensor_shape), mb.dt.np(alloc.dtype)))
        assert nc.dbg_addr is None
        part_name = nc.partition_id_tensor.name if nc.partition_id_tensor else None
        if part_name is not None:
            in_names = [n for n in in_names if n != part_name]
        self.in_names, self.out_names, self.out_avals = in_names, out_names, out_avals
        n_params = len(in_names)
        all_names = in_names + out_names
        if part_name is not None:
            all_names = all_names + [part_name]

        def _body(*args):
            operands = list(args)
            if part_name is not None:
                operands.append(bass2jax.partition_id_tensor())
            outs = bass2jax._bass_exec_p.bind(
                *operands,
                out_avals=tuple(out_avals),
                in_names=tuple(all_names),
                out_names=tuple(out_names),
                lowering_input_output_aliases=(),
                sim_require_finite=True,
                sim_require_nnan=True,
                nc=nc,
            )
            return tuple(outs)

        devices = jax.devices()[:B]
        self.mesh = Mesh(np.asarray(devices), ("core",))
        self.spec = PartitionSpec("core")
        self.sharding = NamedSharding(self.mesh, self.spec)
        nin = n_params + len(out_names)
        self.fn = jax.jit(
            shard_map(
                _body, mesh=self.mesh,
                in_specs=(self.spec,) * nin,
                out_specs=(self.spec,) * len(out_names),
                check_rep=False,
            ),
            donate_argnums=tuple(range(n_params, nin)),
            keep_unused=True,
        )
        self._dev_inputs = {}

    def prep_inputs(self, in_maps, cache_key=None):
        """Concat per-core inputs to global arrays, optionally device-cached."""
        import jax
        if cache_key is not None and cache_key in self._dev_inputs:
            return self._dev_inputs[cache_key]
        concat = [
            np.concatenate([np.asarray(m[n]) for m in in_maps], axis=0)
            for n in self.in_names
        ]
        arrs = [jax.device_put(a, self.sharding) for a in concat]
        jax.block_until_ready(arrs)
        if cache_key is not None:
            self._dev_inputs[cache_key] = arrs
        return arrs

    def __call__(self, dev_inputs, block=True):
        import jax
        zeros = [
            np.zeros((B * av.shape[0], *av.shape[1:]), av.dtype)
            for av in self.out_avals
        ]
        outs = self.fn(*dev_inputs, *zeros)
        if block:
            jax.block_until_ready(outs)
        return outs

    def gather(self, outs):
        o = np.asarray(outs[0])
        return o.reshape(B, -1, o.shape[-1])


def _get_runner(mm_dtype: str, repeat: int) -> "_Runner":
    key = (mm_dtype, repeat)
    if key not in _NC_CACHE:
        _NC_CACHE[key] = _Runner(build_attention_nc(mm_dtype, repeat))
    return _NC_CACHE[key]


def _make_in_maps(inputs: dict):
    x = np.asarray(inputs["x"], dtype=np.float32)
    wq = np.ascontiguousarray(np.asarray(inputs["Wq"], dtype=np.float32))
    wk = np.ascontiguousarray(np.asarray(inputs["Wk"], dtype=np.float32))
    wv = np.ascontiguousarray(np.asarray(inputs["Wv"], dtype=np.float32))
    return [
        {"x": np.ascontiguousarray(x[i]), "Wq": wq, "Wk": wk, "Wv": wv}
        for i in range(B)
    ]


def run_spmd(inputs: dict, mm_dtype: str = MM_DTYPE, repeat: int = 1,
             cache_key=None):
    r = _get_runner(mm_dtype, repeat)
    dev = r.prep_inputs(_make_in_maps(inputs), cache_key=cache_key)
    return r.gather(r(dev))


def kernel(**inputs) -> np.ndarray:
    return run_spmd(inputs, MM_DTYPE, repeat=1)
